# revision 1
# baseline (speedup 1.0000x reference)
"""MultiHeadAttention (B=2, S=2048, D=2048, H=16, RoPE) on 8 NeuronCores.

Sharding: tensor-parallel over heads. Core c owns heads 2c, 2c+1 (256 channels).
Each core: QKV projections for its channels, RoPE, full attention for its 2
heads, and a partial output projection y_c = ctx_c @ Wo[:, ch_c].T. Host sums
the 8 partials.

Dataflow (all matmuls fp32r = full-rate fp32 on PE, ~1e-4 rel):
  - host pre-transposes x -> xT [D, B*S] and weights so every matmul operand
    has its contraction dim on partitions; no on-chip transposes at all.
  - qT/kT produced in [head_dim, token] layout; v in [token, ch] layout.
  - scores computed transposed: scoresT[ktok, qtok] = k @ q.T via
    stationary kT-tile; exp on ScalarE (no max subtraction needed: scores
    are ~N(0,1), fp32 exp is safe); PV accumulates ctxT[hd, qtok] with
    stationary v-tile; softmax denominator via ones-vector matmul; the
    1/denom row is partition-broadcast with a K=1 matmul.
  - output projection consumes ctxT directly as the stationary operand.
"""
import sys

sys.path.insert(0, "/opt/trn_rl_repo")

import numpy as np

B, S, D, H = 2, 2048, 2048, 16
HD = D // H          # 128
NCORES = 8
HPC = H // NCORES    # heads per core
CPC = HPC * HD       # channels per core = 256
TOK = B * S          # 4096
P = 128
KT = D // P          # 16 contraction tiles
NCH = 512            # token chunk for projections / attention qtok chunk
ROPE_BASE = 10000.0

_cache = {}


def _build_nc():
    import concourse.bass as bass  # noqa: F401
    import concourse.mybir as mybir
    import concourse.tile as tile
    from concourse import bacc

    F32 = mybir.dt.float32
    F32R = mybir.dt.float32r
    AF = mybir.ActivationFunctionType
    MUL = mybir.AluOpType.mult
    ADD = mybir.AluOpType.add

    def r(ap):
        return ap

    nc = bacc.Bacc(None, target_bir_lowering=False)

    xT_d = nc.dram_tensor("xT", [D, TOK], F32R, kind="ExternalInput")
    wq_d = nc.dram_tensor("wqT", [D, CPC], F32R, kind="ExternalInput")
    wk_d = nc.dram_tensor("wkT", [D, CPC], F32R, kind="ExternalInput")
    wv_d = nc.dram_tensor("wvT", [D, CPC], F32R, kind="ExternalInput")
    wo_d = nc.dram_tensor("woT", [CPC, D], F32R, kind="ExternalInput")
    cos_d = nc.dram_tensor("cos2", [P, S], F32, kind="ExternalInput")
    sin_d = nc.dram_tensor("sin2", [P, S], F32, kind="ExternalInput")
    y_d = nc.dram_tensor("y", [TOK, D], F32, kind="ExternalOutput")

    SCALE = 1.0 / float(np.sqrt(HD))
    NQC = TOK // NCH            # 8 projection chunks
    SQC = S // NCH              # 4 attention q-chunks per sequence
    NCHA = NCH // 2             # attention q-chunk (256) for double buffering
    SKT = S // P                # 16 key tiles per sequence
    VST = NCH // P              # 4 v subtiles per chunk

    with tile.TileContext(nc) as tc, \
         nc.allow_low_precision(reason="fp32r is bitwise fp32 on the wire"):
        with tc.tile_pool(name="qkv", bufs=1) as qkv_pool:
            # long-lived tensors
            qT = [qkv_pool.tile([P, TOK], F32R, name=f"qT{m}") for m in range(HPC)]
            kTt = [qkv_pool.tile([P, TOK], F32R, name=f"kT{m}") for m in range(HPC)]
            vS = qkv_pool.tile([P, TOK // P, CPC], F32R, name="vS")

            # ---------------- Phase 1: projections + RoPE ----------------
            with tc.tile_pool(name="wp", bufs=1) as wp, \
                 tc.tile_pool(name="xp", bufs=3) as xp, \
                 tc.tile_pool(name="rp", bufs=2) as rp, \
                 tc.tile_pool(name="pp", bufs=1, space="PSUM") as pp:
                wq = wp.tile([P, KT, CPC], F32R, name="wq")
                wk = wp.tile([P, KT, CPC], F32R, name="wk")
                wv = wp.tile([P, KT, CPC], F32R, name="wv")
                nc.sync.dma_start(wq[:], wq_d.rearrange("(o p) c -> p o c", p=P))
                nc.sync.dma_start(wk[:], wk_d.rearrange("(o p) c -> p o c", p=P))
                nc.sync.dma_start(wv[:], wv_d.rearrange("(o p) c -> p o c", p=P))
                cos2 = wp.tile([P, S], F32, name="cos2")
                sin2 = wp.tile([P, S], F32, name="sin2")
                nc.sync.dma_start(cos2[:], cos_d[:])
                nc.sync.dma_start(sin2[:], sin_d[:])

                for ch in range(NQC):
                    t0 = ch * NCH
                    s0 = (ch % SQC) * NCH  # position within sequence
                    q_ps = [pp.tile([P, NCH], F32, name=f"qps{m}") for m in range(HPC)]
                    k_ps = [pp.tile([P, NCH], F32, name=f"kps{m}") for m in range(HPC)]
                    v_ps = [pp.tile([P, CPC], F32, name=f"vps{st}") for st in range(VST)]
                    for kt in range(KT):
                        xt = xp.tile([P, NCH], F32R, name="xt")
                        nc.sync.dma_start(
                            xt[:], xT_d[kt * P:(kt + 1) * P, t0:t0 + NCH]
                        )
                        st_, sp_ = (kt == 0), (kt == KT - 1)
                        for m in range(HPC):
                            nc.tensor.matmul(
                                q_ps[m][:], r(wq[:, kt, m * P:(m + 1) * P]), r(xt[:]),
                                start=st_, stop=sp_,
                            )
                            nc.tensor.matmul(
                                k_ps[m][:], r(wk[:, kt, m * P:(m + 1) * P]), r(xt[:]),
                                start=st_, stop=sp_,
                            )
                        for st in range(VST):
                            nc.tensor.matmul(
                                v_ps[st][:], r(xt[:, st * P:(st + 1) * P]),
                                r(wv[:, kt, :]),
                                start=st_, stop=sp_,
                            )
                    # RoPE: out = q*cos2 + swaphalves(q)*sin2
                    HF = HD // 2
                    for m in range(HPC):
                        for src_ps, dst in ((q_ps[m], qT[m]), (k_ps[m], kTt[m])):
                            rot = rp.tile([P, NCH], F32, name="rot")
                            nc.vector.tensor_copy(rot[0:HF, :], src_ps[HF:P, :])
                            nc.vector.tensor_copy(rot[HF:P, :], src_ps[0:HF, :])
                            nc.vector.tensor_tensor(
                                rot[:], rot[:], sin2[:, s0:s0 + NCH], MUL
                            )
                            tmp = rp.tile([P, NCH], F32, name="tmp")
                            nc.vector.tensor_tensor(
                                tmp[:], src_ps[:], cos2[:, s0:s0 + NCH], MUL
                            )
                            nc.vector.tensor_tensor(
                                dst[:, t0:t0 + NCH], tmp[:], rot[:], ADD
                            )
                    for st in range(VST):
                        gst = ch * VST + st
                        nc.vector.tensor_copy(vS[:, gst, :], v_ps[st][:])

            # ---------------- Phase 2+3: attention + output projection ----
            with tc.tile_pool(name="op", bufs=1) as op, \
                 tc.tile_pool(name="ep", bufs=2) as ep, \
                 tc.tile_pool(name="dp", bufs=2) as dp, \
                 tc.tile_pool(name="yp", bufs=3) as yp, \
                 tc.tile_pool(name="sp2", bufs=3, space="PSUM") as spsum, \
                 tc.tile_pool(name="ap", bufs=2, space="PSUM") as apsum, \
                 tc.tile_pool(name="cp", bufs=1, space="PSUM") as cpsum:
                ctxT = [
                    op.tile([P, S], F32R, name=f"ctxT{b}_{m}")
                    for b in range(B)
                    for m in range(HPC)
                ]
                wo = op.tile([P, HPC, D], F32R, name="wo")
                nc.sync.dma_start(wo[:], wo_d.rearrange("(o p) c -> p o c", p=P))
                ones_col = op.tile([P, 1], F32R, name="ones_col")
                ones_row = op.tile([1, P], F32R, name="ones_row")
                ones_f32c = op.tile([P, 1], F32, name="ones_f32c")
                ones_f32r_ = op.tile([1, P], F32, name="ones_f32r_")
                nc.vector.memset(ones_f32c[:], 1.0)
                nc.vector.memset(ones_f32r_[:], 1.0)
                nc.vector.tensor_copy(ones_col[:], ones_f32c[:])
                nc.vector.tensor_copy(ones_row[:], ones_f32r_[:])

                for b in range(B):
                    for m in range(HPC):
                        pair = b * HPC + m
                        for qc in range(SQC * 2):
                            qt0 = b * S + qc * NCHA
                            ex = ep.tile([P, SKT, NCHA], F32R, name="ex")
                            for kt in range(SKT):
                                scr = spsum.tile([P, NCHA], F32, name="scr")
                                nc.tensor.matmul(
                                    scr[:],
                                    r(kTt[m][:, b * S + kt * P: b * S + (kt + 1) * P]),
                                    r(qT[m][:, qt0:qt0 + NCHA]),
                                    start=True, stop=True,
                                )
                                nc.scalar.activation(
                                    ex[:, kt, :], scr[:], AF.Exp, scale=SCALE
                                )
                            ctx_ps = cpsum.tile([P, NCHA], F32, name="ctx_ps")
                            den_ps = cpsum.tile([1, NCHA], F32, name="den_ps")
                            for kt in range(SKT):
                                gkt = b * SKT + kt
                                st_, sp_ = (kt == 0), (kt == SKT - 1)
                                nc.tensor.matmul(
                                    ctx_ps[:],
                                    r(vS[:, gkt, m * P:(m + 1) * P]),
                                    r(ex[:, kt, :]),
                                    start=st_, stop=sp_,
                                )
                                nc.tensor.matmul(
                                    den_ps[:], r(ones_col[:]), r(ex[:, kt, :]),
                                    start=st_, stop=sp_,
                                )
                            rec = dp.tile([1, NCHA], F32R, name="rec")
                            nc.vector.reciprocal(rec[:], den_ps[:])
                            bc_ps = cpsum.tile([P, NCHA], F32, name="bc_ps")
                            nc.tensor.matmul(
                                bc_ps[:], r(ones_row[:]), r(rec[:]),
                                start=True, stop=True,
                            )
                            bc_sb = dp.tile([P, NCHA], F32, name="bc_sb")
                            nc.vector.tensor_copy(bc_sb[:], bc_ps[:])
                            nc.vector.tensor_tensor(
                                ctxT[pair][:, qc * NCHA:(qc + 1) * NCHA],
                                ctx_ps[:], bc_sb[:], MUL,
                            )
                    # output projection for this b (both heads' ctxT ready)
                    for tt in range(S // P):
                        row0 = b * S + tt * P
                        for nck in range(D // NCH):
                            y_ps = apsum.tile([P, NCH], F32, name="y_ps")
                            for m in range(HPC):
                                nc.tensor.matmul(
                                    y_ps[:],
                                    r(ctxT[b * HPC + m][:, tt * P:(tt + 1) * P]),
                                    r(wo[:, m, nck * NCH:(nck + 1) * NCH]),
                                    start=(m == 0), stop=(m == HPC - 1),
                                )
                            y_sb = yp.tile([P, NCH], F32, name="y_sb")
                            nc.vector.tensor_copy(y_sb[:], y_ps[:])
                            nc.sync.dma_start(
                                y_d[row0:row0 + P, nck * NCH:(nck + 1) * NCH],
                                y_sb[:],
                            )
    nc.finalize()
    return nc


def _rope_tables():
    inv_freq = (1.0 / (ROPE_BASE ** (np.arange(0, HD, 2, dtype=np.float32) / HD))).astype(np.float32)
    t = np.arange(S, dtype=np.float32)
    freqs = np.outer(t, inv_freq).astype(np.float32)  # [S, HD/2]
    c = np.cos(freqs).astype(np.float32).T            # [64, S]
    s = np.sin(freqs).astype(np.float32).T
    cos2 = np.concatenate([c, c], axis=0)             # [128, S]
    sin2 = np.concatenate([-s, s], axis=0)            # [128, S]
    return np.ascontiguousarray(cos2), np.ascontiguousarray(sin2)


def kernel(x, Wq, Wk, Wv, Wo):
    from concourse.bass_utils import run_bass_kernel_spmd

    x = np.asarray(x, dtype=np.float32)
    Wq = np.asarray(Wq, dtype=np.float32)
    Wk = np.asarray(Wk, dtype=np.float32)
    Wv = np.asarray(Wv, dtype=np.float32)
    Wo = np.asarray(Wo, dtype=np.float32)

    xT = np.ascontiguousarray(x.reshape(TOK, D).T)    # [D, TOK]
    cos2, sin2 = _rope_tables()

    in_maps = []
    for c in range(NCORES):
        ch0, ch1 = c * CPC, (c + 1) * CPC
        in_maps.append({
            "xT": xT,
            "wqT": np.ascontiguousarray(Wq[ch0:ch1, :].T),
            "wkT": np.ascontiguousarray(Wk[ch0:ch1, :].T),
            "wvT": np.ascontiguousarray(Wv[ch0:ch1, :].T),
            "woT": np.ascontiguousarray(Wo[:, ch0:ch1].T),
            "cos2": cos2,
            "sin2": sin2,
        })

    if "nc" not in _cache:
        _cache["nc"] = _build_nc()
    res = run_bass_kernel_spmd(_cache["nc"], in_maps, core_ids=list(range(NCORES)))
    _cache["last_results"] = res

    y = np.zeros((TOK, D), dtype=np.float32)
    for rm in res.results:
        y += rm["y"]
    return y.reshape(B, S, D)



# revision 5
# speedup vs baseline: 1.5451x; 1.5451x over previous
"""MultiHeadAttention (B=2, S=2048, D=2048, H=16, RoPE) on 8 NeuronCores.

Sharding: tensor-parallel over heads. Core c owns heads 2c, 2c+1 (256 channels).
Each core: QKV projections for its channels, RoPE, full attention for its 2
heads, and a partial output projection y_c = ctx_c @ Wo[:, ch_c].T. Host sums
the 8 partials (bf16 partials, fp32 host accumulation).

All matmuls bf16 (full-rate streaming + FWL fast weight loads).

Phase 1 (projections + RoPE), per 512-token chunk:
  - q-pass (2 PSUM banks), k-pass (2 banks), v-pass (4 banks) staggered so
    RoPE/DVE drains one bank group while the PE streams the next.
Phase 2 (attention + output projection), per 512-query chunk, per head:
  - scores into 2-bank PSUM groups; one exp per group on ScalarE (bf16 out).
  - softmax denominator: pairwise adds of exp tiles split across DVE/GpSimd,
    gpsimd.partition_all_reduce (reduce + broadcast in one op),
    reciprocal_approx_fast on DVE, then one DVE multiply -> normalized ctxT.
    No PE matmuls are spent on the denominator or broadcast.
  - output projection is software-pipelined one chunk behind and interleaved
    into the PE instruction stream between score-group slots.
"""
import sys

sys.path.insert(0, "/opt/trn_rl_repo")

import numpy as np

B, S, D, H = 2, 2048, 2048, 16
HD = D // H          # 128
NCORES = 8
HPC = H // NCORES    # heads per core = 2
CPC = HPC * HD       # channels per core = 256
TOK = B * S          # 4096
P = 128
KT = D // P          # 16 contraction tiles
NCH = 512            # phase-1 token chunk
NQC = TOK // NCH     # 8 projection chunks
QC = 512             # attention q chunk
SQC = S // QC        # 4 q chunks per sequence
SKT = S // P         # 16 key tiles per sequence
NG = SKT // 2        # 8 key-tile pair groups
VST = NCH // P       # 4 v subtiles per chunk
HF = HD // 2         # 64
ROPE_BASE = 10000.0

_cache = {}


def _build_nc():
    import concourse.bass as bass  # noqa: F401
    import concourse.mybir as mybir
    import concourse.tile as tile
    from concourse import bacc
    from concourse import bass_isa

    F32 = mybir.dt.float32
    BF16 = mybir.dt.bfloat16
    AF = mybir.ActivationFunctionType
    MUL = mybir.AluOpType.mult
    ADD = mybir.AluOpType.add

    nc = bacc.Bacc(None, target_bir_lowering=False)

    xT_d = nc.dram_tensor("xT", [D, TOK], BF16, kind="ExternalInput")
    wq_d = nc.dram_tensor("wqT", [D, CPC], BF16, kind="ExternalInput")
    wk_d = nc.dram_tensor("wkT", [D, CPC], BF16, kind="ExternalInput")
    wv_d = nc.dram_tensor("wvT", [D, CPC], BF16, kind="ExternalInput")
    wo_d = nc.dram_tensor("woT", [CPC, D], BF16, kind="ExternalInput")
    cos_d = nc.dram_tensor("cos2", [P, S], BF16, kind="ExternalInput")
    sin_d = nc.dram_tensor("sin2", [P, S], BF16, kind="ExternalInput")
    y_d = nc.dram_tensor("y", [TOK, D], BF16, kind="ExternalOutput")

    SCALE = 1.0 / float(np.sqrt(HD))
    NCK = D // NCH        # 4 output-channel chunks
    NTT = QC // P         # 4 token tiles per attention chunk

    with tile.TileContext(nc) as tc, \
         nc.allow_low_precision(reason="bf16 everywhere; validated vs fp32 ref"):
        with tc.tile_pool(name="per", bufs=1) as per:
            qT = [per.tile([P, TOK], BF16, name=f"qT{m}") for m in range(HPC)]
            kT = [per.tile([P, TOK], BF16, name=f"kT{m}") for m in range(HPC)]
            vS = per.tile([P, TOK // P, CPC], BF16, name="vS")

            # ---------------- Phase 1: projections + RoPE ----------------
            with tc.tile_pool(name="wp", bufs=1) as wp, \
                 tc.tile_pool(name="xp", bufs=2) as xp, \
                 tc.tile_pool(name="rp", bufs=3) as rp, \
                 tc.tile_pool(name="pq", bufs=1, space="PSUM") as pqp, \
                 tc.tile_pool(name="pk", bufs=1, space="PSUM") as pkp, \
                 tc.tile_pool(name="pv", bufs=1, space="PSUM") as pvp:
                wq = wp.tile([P, KT, CPC], BF16, name="wq")
                nc.sync.dma_start(wq[:], wq_d.rearrange("(o p) c -> p o c", p=P))
                wk = wp.tile([P, KT, CPC], BF16, name="wk")
                wv = wp.tile([P, KT, CPC], BF16, name="wv")
                cos2 = wp.tile([P, S], BF16, name="cos2")
                sin2 = wp.tile([P, S], BF16, name="sin2")
                nc.sync.dma_start(wk[:], wk_d.rearrange("(o p) c -> p o c", p=P))
                nc.sync.dma_start(wv[:], wv_d.rearrange("(o p) c -> p o c", p=P))
                nc.sync.dma_start(cos2[:], cos_d[:])
                nc.sync.dma_start(sin2[:], sin_d[:])

                xT_r = xT_d.rearrange("(o p) t -> p o t", p=P)

                def rope(src_ps, dst, t0, s0):
                    # dst[:, t0:t0+NCH] = src*cos2 + swaphalves(src)*sin2
                    rot = rp.tile([P, NCH], BF16, name="rot")
                    nc.vector.tensor_copy(rot[0:HF, :], src_ps[HF:P, :])
                    nc.vector.tensor_copy(rot[HF:P, :], src_ps[0:HF, :])
                    tmp = rp.tile([P, NCH], BF16, name="tmp")
                    nc.vector.tensor_tensor(
                        tmp[:], src_ps, cos2[:, s0:s0 + NCH], MUL
                    )
                    rs = rp.tile([P, NCH], BF16, name="rs")
                    nc.vector.tensor_tensor(
                        rs[:], rot[:], sin2[:, s0:s0 + NCH], MUL
                    )
                    nc.vector.tensor_tensor(
                        dst[:, t0:t0 + NCH], tmp[:], rs[:], ADD
                    )

                for ch in range(NQC):
                    t0 = ch * NCH
                    s0 = (ch % SQC) * NCH  # position within sequence
                    xc = xp.tile([P, KT, NCH], BF16, name="xc")
                    nc.sync.dma_start(xc[:], xT_r[:, :, t0:t0 + NCH])

                    q_ps = pqp.tile([P, HPC, NCH], F32, name="q_ps")
                    for kt in range(KT):
                        for m in range(HPC):
                            nc.tensor.matmul(
                                q_ps[:, m, :], wq[:, kt, m * P:(m + 1) * P],
                                xc[:, kt, :],
                                start=(kt == 0), stop=(kt == KT - 1),
                            )
                    for m in range(HPC):
                        rope(q_ps[:, m, :], qT[m], t0, s0)

                    k_ps = pkp.tile([P, HPC, NCH], F32, name="k_ps")
                    for kt in range(KT):
                        for m in range(HPC):
                            nc.tensor.matmul(
                                k_ps[:, m, :], wk[:, kt, m * P:(m + 1) * P],
                                xc[:, kt, :],
                                start=(kt == 0), stop=(kt == KT - 1),
                            )
                    for m in range(HPC):
                        rope(k_ps[:, m, :], kT[m], t0, s0)

                    # v: [tok, ch] layout; each subtile gets its own bank
                    v_ps = pvp.tile([P, VST, NCH], F32, name="v_ps")
                    for kt in range(KT):
                        for st in range(VST):
                            nc.tensor.matmul(
                                v_ps[:, st, 0:CPC],
                                xc[:, kt, st * P:(st + 1) * P],
                                wv[:, kt, :],
                                start=(kt == 0), stop=(kt == KT - 1),
                            )
                    for st in range(VST):
                        nc.scalar.copy(
                            vS[:, ch * VST + st, :], v_ps[:, st, 0:CPC]
                        )

            # ---------------- Phase 2: attention + output projection -----
            with tc.tile_pool(name="op", bufs=1) as op, \
                 tc.tile_pool(name="ep", bufs=2) as ep, \
                 tc.tile_pool(name="ap", bufs=4) as app, \
                 tc.tile_pool(name="dp", bufs=2) as dp, \
                 tc.tile_pool(name="cxp", bufs=2) as cxp, \
                 tc.tile_pool(name="yp", bufs=3) as yp, \
                 tc.tile_pool(name="sp", bufs=2, space="PSUM") as spsum, \
                 tc.tile_pool(name="cp", bufs=2, space="PSUM") as cpsum, \
                 tc.tile_pool(name="yq", bufs=1, space="PSUM") as ypsum:
                wo = op.tile([P, HPC, D], BF16, name="wo")
                nc.sync.dma_start(wo[:], wo_d.rearrange("(m p) d -> p m d", p=P))

                def outproj_steps(ctx_pair, b_p, qc_p):
                    # 8 steps; each: 4 matmuls into a 2-bank PSUM tile,
                    # one DVE copy, one DMA of [128, 1024] bf16.
                    steps = []
                    for tt in range(NTT):
                        for npair in range(NCK // 2):
                            idx = tt * (NCK // 2) + npair

                            def step(tt=tt, npair=npair, idx=idx):
                                y_ps = ypsum.tile([P, 2, NCH], F32, name="y_ps")
                                for j in range(2):
                                    nck = 2 * npair + j
                                    for m in range(HPC):
                                        nc.tensor.matmul(
                                            y_ps[:, j, :],
                                            ctx_pair[m][:, tt * P:(tt + 1) * P],
                                            wo[:, m, nck * NCH:(nck + 1) * NCH],
                                            start=(m == 0), stop=(m == HPC - 1),
                                        )
                                y_sb = yp.tile([P, 2, NCH], BF16, name="y_sb")
                                if idx % 4 == 3:
                                    nc.scalar.copy(y_sb[:], y_ps[:])
                                else:
                                    nc.vector.tensor_copy(y_sb[:], y_ps[:])
                                row0 = b_p * S + qc_p * QC + tt * P
                                c0 = 2 * npair * NCH
                                nc.sync.dma_start(
                                    y_d[row0:row0 + P, c0:c0 + 2 * NCH], y_sb[:]
                                )
                            steps.append(step)
                    return steps

                prev_steps = []
                for b in range(B):
                    for qc in range(SQC):
                        qt0 = b * S + qc * QC
                        ctxT = [
                            cxp.tile([P, QC], BF16, name=f"ctxT{m}")
                            for m in range(HPC)
                        ]
                        pending = list(prev_steps)
                        pi = 0
                        slot = 0
                        for m in range(HPC):
                            ex = ep.tile([P, NG, 2, QC], BF16, name="ex")
                            ctx_ps = cpsum.tile([P, QC], F32, name="ctx_ps")
                            lvl1 = []

                            def pv(g, h):
                                gk = b * SKT + 2 * g + h
                                first = (g == 0 and h == 0)
                                last = (g == NG - 1 and h == 1)
                                nc.tensor.matmul(
                                    ctx_ps[:],
                                    vS[:, gk, m * P:(m + 1) * P],
                                    ex[:, g, h, :],
                                    start=first, stop=last,
                                )

                            for g in range(NG):
                                scr = spsum.tile([P, 2, QC], F32, name="scr")
                                for h in range(2):
                                    k0 = b * S + (2 * g + h) * P
                                    nc.tensor.matmul(
                                        scr[:, h, :],
                                        kT[m][:, k0:k0 + P],
                                        qT[m][:, qt0:qt0 + QC],
                                        start=True, stop=True,
                                    )
                                nc.scalar.activation(
                                    ex[:, g, :, :], scr[:], AF.Exp, scale=SCALE
                                )
                                if g >= 2:
                                    pv(g - 2, 0)
                                    pv(g - 2, 1)
                                if slot >= 2 and pi < len(pending):
                                    pending[pi]()
                                    pi += 1
                                slot += 1
                                # denominator: wide pairwise adds over
                                # adjacent exp groups ([P, 2, QC] each)
                                if g % 2 == 1:
                                    pd = app.tile([P, 2, QC], BF16, name="pd")
                                    nc.vector.tensor_tensor(
                                        pd[:], ex[:, g - 1, :, :],
                                        ex[:, g, :, :], ADD,
                                    )
                                    lvl1.append(pd)
                            for gg in (NG - 2, NG - 1):
                                pv(gg, 0)
                                pv(gg, 1)
                            l2a = app.tile([P, 2, QC], BF16, name="l2a")
                            nc.vector.tensor_tensor(
                                l2a[:], lvl1[0][:], lvl1[1][:], ADD
                            )
                            l2b = app.tile([P, 2, QC], BF16, name="l2b")
                            nc.vector.tensor_tensor(
                                l2b[:], lvl1[2][:], lvl1[3][:], ADD
                            )
                            l3 = app.tile([P, 2, QC], BF16, name="l3")
                            nc.vector.tensor_tensor(l3[:], l2a[:], l2b[:], ADD)
                            exs = dp.tile([P, QC], BF16, name="exs")
                            nc.vector.tensor_tensor(
                                exs[:], l3[:, 0, :], l3[:, 1, :], ADD
                            )
                            den = dp.tile([P, QC], F32, name="den")
                            nc.gpsimd.partition_all_reduce(
                                den[:], exs[:], channels=P,
                                reduce_op=bass_isa.ReduceOp.add,
                            )
                            rec = dp.tile([P, QC], F32, name="rec")
                            nc.vector.reciprocal_approx_fast(rec[:], den[:])
                            nc.vector.tensor_tensor(
                                ctxT[m][:], ctx_ps[:], rec[:], MUL
                            )
                        while pi < len(pending):
                            pending[pi]()
                            pi += 1
                        prev_steps = outproj_steps(ctxT, b, qc)
                for st_fn in prev_steps:
                    st_fn()
    nc.finalize()
    return nc


def _rope_tables():
    inv_freq = (1.0 / (ROPE_BASE ** (np.arange(0, HD, 2, dtype=np.float32) / HD))).astype(np.float32)
    t = np.arange(S, dtype=np.float32)
    freqs = np.outer(t, inv_freq).astype(np.float32)  # [S, HD/2]
    c = np.cos(freqs).astype(np.float32).T            # [64, S]
    s = np.sin(freqs).astype(np.float32).T
    cos2 = np.concatenate([c, c], axis=0)             # [128, S]
    sin2 = np.concatenate([-s, s], axis=0)            # [128, S]
    return np.ascontiguousarray(cos2), np.ascontiguousarray(sin2)


def kernel(x, Wq, Wk, Wv, Wo):
    import ml_dtypes
    from concourse.bass_utils import run_bass_kernel_spmd

    BF = ml_dtypes.bfloat16
    x = np.asarray(x, dtype=np.float32)
    Wq = np.asarray(Wq, dtype=np.float32)
    Wk = np.asarray(Wk, dtype=np.float32)
    Wv = np.asarray(Wv, dtype=np.float32)
    Wo = np.asarray(Wo, dtype=np.float32)

    xT = np.ascontiguousarray(x.reshape(TOK, D).T).astype(BF)  # [D, TOK]
    cos2, sin2 = _rope_tables()
    cos2 = cos2.astype(BF)
    sin2 = sin2.astype(BF)

    in_maps = []
    for c in range(NCORES):
        ch0, ch1 = c * CPC, (c + 1) * CPC
        in_maps.append({
            "xT": xT,
            "wqT": np.ascontiguousarray(Wq[ch0:ch1, :].T).astype(BF),
            "wkT": np.ascontiguousarray(Wk[ch0:ch1, :].T).astype(BF),
            "wvT": np.ascontiguousarray(Wv[ch0:ch1, :].T).astype(BF),
            "woT": np.ascontiguousarray(Wo[:, ch0:ch1].T).astype(BF),
            "cos2": cos2,
            "sin2": sin2,
        })

    if "nc" not in _cache:
        _cache["nc"] = _build_nc()
    res = run_bass_kernel_spmd(_cache["nc"], in_maps, core_ids=list(range(NCORES)))
    _cache["last_results"] = res

    y = np.zeros((TOK, D), dtype=np.float32)
    for rm in res.results:
        y += np.asarray(rm["y"], dtype=np.float32)
    return y.reshape(B, S, D)
